# revision 5
# baseline (speedup 1.0000x reference)
"""Paged GQA chunked-prefill attention for 8 Trainium2 NeuronCores.

Problem (hardcoded): B=4 seqs x Q=256 new tokens, H=32 query heads, KVH=8 kv
heads (GQA group G=4), D=128 head dim, paged KV cache of 512 blocks x 16
tokens, per-seq lengths in seq_lens (clamped to >= Q), causal masking.

Sharding: tensor-parallel over heads. Core h gets kv head h and query heads
h*4..h*4+3; block_tables/seq_lens are resolved host-side while packing the
shards; the output is all-gathered host-side over the hidden dim.

v2 design (per core; q = (t, g) -> 1024 columns/seq; kv chunks of 128):
  S^T[kv,q] = K_c^T q          fp16 matmul into PSUM (full PE rate)
  u = exp(SCALE*S^T)           ScalarE, PSUM->SBUF, fp16 out
  mask                         multiplicative 0/1 fp16 band tiles on VectorE
  l_acc += u                   VectorE fp16 adds (4x perf mode) -- keeps the
                               denominator reduction OFF the PE
  O^T += V_c^T u               fp16 matmul, PSUM accumulation over chunks
Per-seq: partition_all_reduce(l_acc) on GpSimd -> l broadcast to all
partitions, reciprocal on VectorE, out = O^T * rl (GpSimd for interleaved
seqs, VectorE halves for the tail seq), DMA out.

Scheduling: the shortest seq runs QK/exp first (smallest DMA critical path)
but its PV runs LAST, so the tail after the final matmul is just one small
PV group + per-half multiply + DMA. Fully-masked query columns are skipped
(shrunk matmul/exp widths). 8 warm-up matmuls on a zeroed tile run during
the input DMA window so the PE HAM clock-gate is at 2.4 GHz when real work
arrives. DMA issues are spread over the SP/ACT/DVE/Pool sequencers.
"""
import math

import numpy as np

import concourse.mybir as mybir
import concourse.tile as tile
from concourse import bacc, bass_isa
from concourse.bass_utils import run_bass_kernel_spmd

B, Q, H, D = 4, 256, 32, 128
KVH = 8
G = H // KVH
BLOCK = 16
NB = 128
KV = NB * BLOCK
NUM_BLOCKS = B * NB
SCALE = 1.0 / math.sqrt(D)
N_CORES = 8
CHUNK = 128
QCOLS = G * Q  # 1024 q columns per sequence per core
NHALF = 512

F32 = mybir.dt.float32
F16 = mybir.dt.float16


def _plan(seq_lens):
    """Chunk counts, processing order, per-(seq,chunk,half) mask geometry."""
    L = np.maximum(np.asarray(seq_lens, dtype=np.int64), Q)
    cb = [int((int(x) + CHUNK - 1) // CHUNK) for x in L]
    first = min(range(B), key=lambda b: (cb[b], b))
    rest = sorted((b for b in range(B) if b != first), key=lambda b: (-cb[b], b))
    porder = [first] + rest
    offs = {}
    o = 0
    for b in porder:
        offs[b] = o
        o += cb[b]
    C = o
    # info[(b,c,n)]: None if the whole half is masked, else dict with
    # qlo (dead leading cols), blo/bhi (mask band col range within the half)
    info = {}
    for b in range(B):
        Lb = int(L[b])
        for c in range(cb[b]):
            for n in range(2):
                lo = Lb - Q + n * CHUNK  # qpos of this half's first column
                if c * CHUNK > lo + CHUNK - 1:
                    info[(b, c, n)] = None
                    continue
                tdead = min(max(c * CHUNK - lo, 0), CHUNK)
                thi = min(max(c * CHUNK + CHUNK - 1 - lo, 0), CHUNK)
                info[(b, c, n)] = dict(qlo=G * tdead, blo=G * tdead, bhi=G * thi)
    masks = []  # (b, c, n, blo, bhi, moff) in processing order
    moff = 0
    for b in porder:
        for c in range(cb[b]):
            for n in range(2):
                st = info[(b, c, n)]
                if st is None or st["bhi"] <= st["blo"]:
                    continue
                masks.append((b, c, n, st["blo"], st["bhi"], moff))
                moff += st["bhi"] - st["blo"]
    last_n = {
        b: [
            min(cb[b] - 1, (int(L[b]) - Q + n * CHUNK + CHUNK - 1) // CHUNK)
            for n in range(2)
        ]
        for b in range(B)
    }
    return dict(L=L, cb=cb, porder=porder, offs=offs, C=C, info=info,
                masks=masks, mtot=moff, last_n=last_n)


def _mask_np(plan):
    m = np.zeros((CHUNK, max(plan["mtot"], 1)), dtype=np.float16)
    p = np.arange(CHUNK)[:, None]
    for (b, c, n, blo, bhi, moff) in plan["masks"]:
        lo = int(plan["L"][b]) - Q + n * CHUNK
        t = np.arange(blo, bhi)[None, :] // G
        m[:, moff:moff + (bhi - blo)] = (c * CHUNK + p <= lo + t).astype(
            np.float16
        )
    return m


def _build(seq_lens):
    plan = _plan(seq_lens)
    L, cb, porder, offs = plan["L"], plan["cb"], plan["porder"], plan["offs"]
    C, info, mtot, last_n = plan["C"], plan["info"], plan["mtot"], plan["last_n"]
    midx = {(b, c, n): (blo, bhi, mo)
            for (b, c, n, blo, bhi, mo) in plan["masks"]}
    mask_np = _mask_np(plan)

    nc = bacc.Bacc(
        "TRN2", target_bir_lowering=False, debug=False, num_devices=N_CORES
    )
    kt_d = nc.dram_tensor("kt", [D, C * CHUNK], F16, kind="ExternalInput")
    v_d = nc.dram_tensor("v", [CHUNK, C * CHUNK], F16, kind="ExternalInput")
    qt_d = nc.dram_tensor("qt", [D, B * QCOLS], F16, kind="ExternalInput")
    out_d = nc.dram_tensor("out", [B, D, QCOLS], F32, kind="ExternalOutput")
    mask_d = nc.inline_tensor(mask_np, name="mask_const")

    exp = mybir.ActivationFunctionType.Exp
    radd = bass_isa.ReduceOp.add
    bf = porder[0]
    qbase = {b: i * QCOLS for i, b in enumerate(porder)}

    def kvcols(b):
        return offs[b] * CHUNK, (offs[b] + cb[b]) * CHUNK

    with tile.TileContext(nc) as tc:
        with (
            tc.tile_pool(name="sbin", bufs=1) as sbin,
            tc.tile_pool(name="sbu", bufs=4) as sbu,
            tc.tile_pool(name="sbe", bufs=2) as sbe,
            tc.tile_pool(name="ps_s", bufs=2, space="PSUM") as ps_s,
            tc.tile_pool(name="ps_o", bufs=2, space="PSUM") as ps_o,
        ):
            kt_t = sbin.tile([D, C * CHUNK], F16, tag="kt")
            v_t = sbin.tile([CHUNK, C * CHUNK], F16, tag="v")
            qt_t = sbin.tile([D, B * QCOLS], F16, tag="qt")
            warm = sbin.tile([CHUNK, NHALF], F16, tag="warm")
            masks_t = sbin.tile([CHUNK, max(mtot, 1)], F16, tag="masks")
            lacc = {
                b: sbin.tile([CHUNK, QCOLS], F16, tag=f"lacc{b}", name=f"lacc{b}")
                for b in range(B)
            }
            rl_f = sbin.tile([CHUNK, QCOLS], F32, tag="rlf")

            nc.vector.memset(warm[:], 0.0)

            # ---- input DMAs, spread across sequencers ----
            k0, k1 = kvcols(bf)
            qb0 = qbase[bf]
            nc.sync.dma_start(kt_t[:, k0:k1], kt_d.ap()[:, k0:k1])
            nc.sync.dma_start(
                qt_t[:, qb0:qb0 + QCOLS], qt_d.ap()[:, qb0:qb0 + QCOLS]
            )
            nc.sync.dma_start(v_t[:, k0:k1], v_d.ap()[:, k0:k1])
            s0 = k1
            sb1 = min(C * CHUNK, s0 + 8 * CHUNK)
            nc.sync.dma_start(kt_t[:, s0:sb1], kt_d.ap()[:, s0:sb1])
            if mtot:
                nc.gpsimd.dma_start(masks_t[:], mask_d.ap())
            nc.scalar.dma_start(v_t[:, s0:sb1], v_d.ap()[:, s0:sb1])
            cut = sb1
            while cut < C * CHUNK:
                hi = min(cut + 8 * CHUNK, C * CHUNK)
                nc.gpsimd.dma_start(kt_t[:, cut:hi], kt_d.ap()[:, cut:hi])
                cut = hi
            if B * QCOLS > QCOLS:
                nc.scalar.dma_start(
                    qt_t[:, QCOLS:], qt_d.ap()[:, QCOLS:]
                )
            cut = sb1
            while cut < C * CHUNK:
                hi = min(cut + 8 * CHUNK, C * CHUNK)
                nc.sync.dma_start(v_t[:, cut:hi], v_d.ap()[:, cut:hi])
                cut = hi

            # ---- PE warm-up (HAM un-throttle) on zeroed tile ----
            wps = ps_s.tile([CHUNK, QCOLS], F32, tag="s")
            for _ in range(8):
                nc.tensor.matmul(
                    wps[:, 0:NHALF], warm[:, 0:CHUNK], warm[:],
                    start=True, stop=True,
                )

            def emit_chunk(b, c, u, o_ps):
                st = [info[(b, c, 0)], info[(b, c, 1)]]
                kc = (offs[b] + c) * CHUNK
                qb = qbase[b]
                s_ps = ps_s.tile([CHUNK, QCOLS], F32, tag="s", name="s")
                for n in range(2):
                    if st[n] is None:
                        continue
                    a = n * NHALF + st[n]["qlo"]
                    z = (n + 1) * NHALF
                    nc.tensor.matmul(
                        s_ps[:, a:z],
                        kt_t[:, kc:kc + CHUNK],
                        qt_t[:, qb + a:qb + z],
                        start=True, stop=True,
                    )
                alo = st[0]["qlo"] if st[0] is not None else NHALF + st[1]["qlo"]
                nc.scalar.activation(
                    u[:, alo:QCOLS], s_ps[:, alo:QCOLS], exp, scale=SCALE
                )
                for n in range(2):
                    mi = midx.get((b, c, n))
                    if mi is None:
                        continue
                    blo, bhi, mo = mi
                    a = n * NHALF + blo
                    w = bhi - blo
                    nc.vector.tensor_mul(
                        u[:, a:a + w], u[:, a:a + w], masks_t[:, mo:mo + w]
                    )
                if c == 0:
                    nc.vector.tensor_copy(lacc[b][:], u[:])
                else:
                    nc.vector.tensor_add(
                        lacc[b][:, alo:], lacc[b][:, alo:], u[:, alo:]
                    )
                if o_ps is not None:
                    for n in range(2):
                        if st[n] is None:
                            continue
                        a = n * NHALF + st[n]["qlo"]
                        z = (n + 1) * NHALF
                        nc.tensor.matmul(
                            o_ps[:, a:z],
                            v_t[:, kc:kc + CHUNK],
                            u[:, a:z],
                            start=c == 0, stop=c == last_n[b][n],
                        )

            # ---- phase A: shortest seq QK/exp only (PV deferred) ----
            u_f = []
            for c in range(cb[bf]):
                u = sbin.tile([CHUNK, QCOLS], F16, tag=f"uf{c}", name=f"uf{c}")
                emit_chunk(bf, c, u, None)
                u_f.append(u)
            lbc_f = sbe.tile([CHUNK, QCOLS], F32, tag="lbc")
            nc.gpsimd.partition_all_reduce(lbc_f[:], lacc[bf][:], 128, radd)
            pending = [(rl_f, lbc_f)]  # recips deferred to avoid DVE stalls

            # ---- phase B: remaining seqs, PV interleaved per chunk ----
            rest = porder[1:]
            epil = []  # (b, o_ps, rl_b) emitted after next seq starts
            for b in rest:
                o_ps = ps_o.tile([D, QCOLS], F32, tag="o", name="o")
                for c in range(cb[b]):
                    uu = sbu.tile([CHUNK, QCOLS], F16, tag="u", name="u")
                    emit_chunk(b, c, uu, o_ps)
                    if c == min(2, cb[b] - 1):
                        for (rr, ll) in pending:
                            nc.vector.reciprocal_approx_fast(rr[:], ll[:])
                        pending = []
                        while epil:
                            eb, eo, erl = epil.pop(0)
                            osb = sbe.tile([D, QCOLS], F32, tag="osb", name="osb")
                            nc.vector.tensor_mul(osb[:], eo[:], erl[:])
                            nc.sync.dma_start(out_d.ap()[eb][:, :], osb[:])
                lbc = sbe.tile([CHUNK, QCOLS], F32, tag="lbc", name="lbc")
                nc.gpsimd.partition_all_reduce(lbc[:], lacc[b][:], 128, radd)
                rl_b = sbe.tile([CHUNK, QCOLS], F32, tag="rl", name="rl")
                pending.append((rl_b, lbc))
                epil.append((b, o_ps, rl_b))
            for (rr, ll) in pending:
                nc.vector.reciprocal_approx_fast(rr[:], ll[:])
            while epil:
                eb, eo, erl = epil.pop(0)
                osb = sbe.tile([D, QCOLS], F32, tag="osb", name="osb")
                nc.vector.tensor_mul(osb[:], eo[:], erl[:])
                nc.sync.dma_start(out_d.ap()[eb][:, :], osb[:])

            # ---- phase C: deferred PV of the shortest seq, halved tail ----
            o_ps = ps_o.tile([D, QCOLS], F32, tag="o", name="o")
            out_fsb = sbe.tile([D, QCOLS], F32, tag="osbf")
            for n in range(2):
                for c in range(cb[bf]):
                    st = info[(bf, c, n)]
                    if st is None:
                        continue
                    a = n * NHALF + st["qlo"]
                    z = (n + 1) * NHALF
                    nc.tensor.matmul(
                        o_ps[:, a:z],
                        v_t[:, (offs[bf] + c) * CHUNK:(offs[bf] + c + 1) * CHUNK],
                        u_f[c][:, a:z],
                        start=c == 0, stop=c == last_n[bf][n],
                    )
                hs = slice(n * NHALF, (n + 1) * NHALF)
                nc.vector.tensor_mul(
                    out_fsb[:, hs], o_ps[:, hs], rl_f[:, hs]
                )
                nc.sync.dma_start(out_d.ap()[bf][:, hs], out_fsb[:, hs])

    nc.compile()
    return nc, plan


def _pack_inputs(query, k_cache, v_cache, block_tables, plan):
    """Gather the paged cache and pack per-core fp16 shards."""
    L, cb, porder, offs, C = (
        plan["L"], plan["cb"], plan["porder"], plan["offs"], plan["C"]
    )
    k_lin = k_cache[block_tables].reshape(B, KV, KVH, D)
    v_lin = v_cache[block_tables].reshape(B, KV, KVH, D)
    kt_all = np.zeros((KVH, D, C * CHUNK), dtype=np.float16)
    v_all = np.zeros((KVH, CHUNK, C * CHUNK), dtype=np.float16)
    for b in range(B):
        Lb, w = int(L[b]), cb[b] * CHUNK
        o0 = offs[b] * CHUNK
        kk = np.zeros((w, KVH, D), dtype=np.float32)
        kk[:Lb] = k_lin[b, :Lb]
        kt_all[:, :, o0:o0 + w] = kk.transpose(1, 2, 0).astype(np.float16)
        vv = np.zeros((w, KVH, D), dtype=np.float32)
        vv[:Lb] = v_lin[b, :Lb]
        v_all[:, :, o0:o0 + w] = (
            vv.reshape(cb[b], CHUNK, KVH, D)
            .transpose(2, 1, 0, 3)
            .reshape(KVH, CHUNK, w)
            .astype(np.float16)
        )
    # query [B,Q,H,D] -> porder-major [KVH, D, B*QCOLS] (t-major, g inner)
    qp = query[np.array(porder)]
    qt_all = (
        qp.transpose(2, 3, 0, 1)
        .reshape(KVH, G, D, B, Q)
        .transpose(0, 2, 3, 4, 1)
        .reshape(KVH, D, B * QCOLS)
        .astype(np.float16)
    )
    return [
        {
            "kt": np.ascontiguousarray(kt_all[h]),
            "v": np.ascontiguousarray(v_all[h]),
            "qt": np.ascontiguousarray(qt_all[h]),
        }
        for h in range(KVH)
    ]


def _unpack_outputs(results):
    """[B,D,QCOLS] per core (O^T, q=(t,g) on cols) -> [B*Q, H*D]."""
    out = np.empty((B * Q, H * D), dtype=np.float32)
    for h, res in enumerate(results):
        o = res["out"].reshape(B, D, Q, G)  # [b, d, t, g]
        o = o.transpose(0, 2, 3, 1).reshape(B * Q, G * D)
        out[:, h * G * D:(h + 1) * G * D] = o
    return out


def kernel(query, k_cache, v_cache, block_tables, seq_lens):
    query = np.asarray(query, dtype=np.float32)
    k_cache = np.asarray(k_cache, dtype=np.float32)
    v_cache = np.asarray(v_cache, dtype=np.float32)
    block_tables = np.asarray(block_tables, dtype=np.int64)
    nc, plan = _build(np.asarray(seq_lens))
    in_maps = _pack_inputs(query, k_cache, v_cache, block_tables, plan)
    res = run_bass_kernel_spmd(nc, in_maps, core_ids=list(range(N_CORES)))
    return _unpack_outputs(res.results)


# revision 8
# speedup vs baseline: 1.2433x; 1.2433x over previous
"""Paged GQA chunked-prefill attention for 8 Trainium2 NeuronCores.

Problem (hardcoded): B=4 seqs x Q=256 new tokens, H=32 query heads, KVH=8 kv
heads (GQA group G=4), D=128 head dim, paged KV cache of 512 blocks x 16
tokens, per-seq lengths in seq_lens (clamped to >= Q), causal masking.

Sharding: tensor-parallel over heads. Core h gets kv head h and query heads
h*4..h*4+3; block_tables/seq_lens are resolved host-side while packing the
shards; the output is all-gathered host-side over the hidden dim.

v2 design (per core; q = (t, g) -> 1024 columns/seq; kv chunks of 128):
  S^T[kv,q] = K_c^T q          fp16 matmul into PSUM (full PE rate)
  u = exp(SCALE*S^T)           ScalarE, PSUM->SBUF, fp16 out
  mask                         multiplicative 0/1 fp16 band tiles on VectorE
  l_acc += u                   VectorE fp16 adds (4x perf mode) -- keeps the
                               denominator reduction OFF the PE
  O^T += V_c^T u               fp16 matmul, PSUM accumulation over chunks
Per-seq: partition_all_reduce(l_acc) on GpSimd -> l broadcast to all
partitions, reciprocal on VectorE, out = O^T * rl (GpSimd for interleaved
seqs, VectorE halves for the tail seq), DMA out.

Scheduling: the shortest seq runs QK/exp first (smallest DMA critical path)
but its PV runs LAST, so the tail after the final matmul is just one small
PV group + per-half multiply + DMA. Fully-masked query columns are skipped
(shrunk matmul/exp widths). 8 warm-up matmuls on a zeroed tile run during
the input DMA window so the PE HAM clock-gate is at 2.4 GHz when real work
arrives. DMA issues are spread over the SP/ACT/DVE/Pool sequencers.
"""
import math

import numpy as np

import concourse.mybir as mybir
import concourse.tile as tile
from concourse import bacc, bass_isa
from concourse.bass_utils import run_bass_kernel_spmd

B, Q, H, D = 4, 256, 32, 128
KVH = 8
G = H // KVH
BLOCK = 16
NB = 128
KV = NB * BLOCK
NUM_BLOCKS = B * NB
SCALE = 1.0 / math.sqrt(D)
N_CORES = 8
CHUNK = 128
QCOLS = G * Q  # 1024 q columns per sequence per core
NHALF = 512

F32 = mybir.dt.float32
F16 = mybir.dt.float16


def _plan(seq_lens):
    """Chunk counts, processing order, per-(seq,chunk,half) mask geometry."""
    L = np.maximum(np.asarray(seq_lens, dtype=np.int64), Q)
    cb = [int((int(x) + CHUNK - 1) // CHUNK) for x in L]
    first = min(range(B), key=lambda b: (cb[b], b))
    rest = sorted((b for b in range(B) if b != first), key=lambda b: (-cb[b], b))
    porder = [first] + rest
    offs = {}
    o = 0
    for b in porder:
        offs[b] = o
        o += cb[b]
    C = o
    # info[(b,c,n)]: None if the whole half is masked, else dict with
    # qlo (dead leading cols), blo/bhi (mask band col range within the half)
    info = {}
    for b in range(B):
        Lb = int(L[b])
        for c in range(cb[b]):
            for n in range(2):
                lo = Lb - Q + n * CHUNK  # qpos of this half's first column
                if c * CHUNK > lo + CHUNK - 1:
                    info[(b, c, n)] = None
                    continue
                tdead = min(max(c * CHUNK - lo, 0), CHUNK)
                thi = min(max(c * CHUNK + CHUNK - 1 - lo, 0), CHUNK)
                info[(b, c, n)] = dict(qlo=G * tdead, blo=G * tdead, bhi=G * thi)
    masks = []  # (b, c, n, blo, bhi, moff) in processing order
    moff = 0
    for b in porder:
        for c in range(cb[b]):
            for n in range(2):
                st = info[(b, c, n)]
                if st is None or st["bhi"] <= st["blo"]:
                    continue
                masks.append((b, c, n, st["blo"], st["bhi"], moff))
                moff += st["bhi"] - st["blo"]
    last_n = {
        b: [
            min(cb[b] - 1, (int(L[b]) - Q + n * CHUNK + CHUNK - 1) // CHUNK)
            for n in range(2)
        ]
        for b in range(B)
    }
    return dict(L=L, cb=cb, porder=porder, offs=offs, C=C, info=info,
                masks=masks, mtot=moff, last_n=last_n)


NEG = -20000.0  # exp(SCALE*(s+NEG)) underflows to exactly 0; fp16-exact


def _mask_np(plan):
    m = np.zeros((CHUNK, max(plan["mtot"], 1)), dtype=np.float16)
    p = np.arange(CHUNK)[:, None]
    for (b, c, n, blo, bhi, moff) in plan["masks"]:
        lo = int(plan["L"][b]) - Q + n * CHUNK
        t = np.arange(blo, bhi)[None, :] // G
        m[:, moff:moff + (bhi - blo)] = np.where(
            c * CHUNK + p <= lo + t, 0.0, NEG
        ).astype(np.float16)
    return m


def _build(seq_lens):
    plan = _plan(seq_lens)
    L, cb, porder, offs = plan["L"], plan["cb"], plan["porder"], plan["offs"]
    C, info, mtot, last_n = plan["C"], plan["info"], plan["mtot"], plan["last_n"]
    midx = {(b, c, n): (blo, bhi, mo)
            for (b, c, n, blo, bhi, mo) in plan["masks"]}
    mask_np = _mask_np(plan)

    nc = bacc.Bacc(
        "TRN2", target_bir_lowering=False, debug=False, num_devices=N_CORES
    )
    kt_d = nc.dram_tensor("kt", [D, C * CHUNK], F16, kind="ExternalInput")
    v_d = nc.dram_tensor("v", [CHUNK, C * CHUNK], F16, kind="ExternalInput")
    qt_d = nc.dram_tensor("qt", [D, B * QCOLS], F16, kind="ExternalInput")
    out_d = nc.dram_tensor("out", [B, D, QCOLS], F32, kind="ExternalOutput")
    mask_d = nc.inline_tensor(mask_np, name="mask_const")
    identb_np = np.eye(CHUNK, dtype=np.float16)
    identb_d = nc.inline_tensor(identb_np, name="identb_const")

    exp = mybir.ActivationFunctionType.Exp
    bf = porder[0]
    qbase = {b: i * QCOLS for i, b in enumerate(porder)}

    def kvcols(b):
        return offs[b] * CHUNK, (offs[b] + cb[b]) * CHUNK

    with tile.TileContext(nc) as tc:
        with (
            tc.tile_pool(name="sbin", bufs=1) as sbin,
            tc.tile_pool(name="sbu", bufs=4) as sbu,
            tc.tile_pool(name="sbe", bufs=2) as sbe,
            tc.tile_pool(name="ps_s", bufs=2, space="PSUM") as ps_s,
            tc.tile_pool(name="ps_o", bufs=2, space="PSUM") as ps_o,
        ):
            kt_t = sbin.tile([D, C * CHUNK], F16, tag="kt")
            v_t = sbin.tile([CHUNK, C * CHUNK], F16, tag="v")
            qt_t = sbin.tile([D, B * QCOLS], F16, tag="qt")
            warm = sbin.tile([CHUNK, NHALF], F16, tag="warm")
            masks_t = sbin.tile([CHUNK, max(mtot, 1)], F16, tag="masks")
            lacc = {
                b: sbin.tile([CHUNK, QCOLS], F16, tag=f"lacc{b}", name=f"lacc{b}")
                for b in range(B)
            }
            rl_f = sbin.tile([CHUNK, QCOLS], F32, tag="rlf")
            identb_t = sbin.tile([CHUNK, CHUNK], F16, tag="identb")
            ones_t = sbin.tile([CHUNK, CHUNK], F16, tag="ones")

            nc.vector.memset(warm[:], 0.0)
            nc.vector.memset(ones_t[:], 1.0)
            nc.gpsimd.dma_start(identb_t[:], identb_d.ap())

            # ---- input DMAs, spread across sequencers ----
            k0, k1 = kvcols(bf)
            qb0 = qbase[bf]
            nc.sync.dma_start(kt_t[:, k0:k1], kt_d.ap()[:, k0:k1])
            nc.sync.dma_start(
                qt_t[:, qb0:qb0 + QCOLS], qt_d.ap()[:, qb0:qb0 + QCOLS]
            )
            nc.sync.dma_start(v_t[:, k0:k1], v_d.ap()[:, k0:k1])
            s0 = k1
            sb1 = min(C * CHUNK, s0 + 8 * CHUNK)
            nc.sync.dma_start(kt_t[:, s0:sb1], kt_d.ap()[:, s0:sb1])
            if mtot:
                nc.gpsimd.dma_start(masks_t[:], mask_d.ap())
            nc.scalar.dma_start(v_t[:, s0:sb1], v_d.ap()[:, s0:sb1])
            cut = sb1
            while cut < C * CHUNK:
                hi = min(cut + 8 * CHUNK, C * CHUNK)
                nc.gpsimd.dma_start(kt_t[:, cut:hi], kt_d.ap()[:, cut:hi])
                cut = hi
            if B * QCOLS > QCOLS:
                nc.scalar.dma_start(
                    qt_t[:, QCOLS:], qt_d.ap()[:, QCOLS:]
                )
            cut = sb1
            while cut < C * CHUNK:
                hi = min(cut + 8 * CHUNK, C * CHUNK)
                nc.sync.dma_start(v_t[:, cut:hi], v_d.ap()[:, cut:hi])
                cut = hi

            # ---- PE warm-up (HAM un-throttle) on zeroed tile ----
            wps = ps_s.tile([CHUNK, QCOLS], F32, tag="s")
            for _ in range(8):
                nc.tensor.matmul(
                    wps[:, 0:NHALF], warm[:, 0:CHUNK], warm[:],
                    start=True, stop=True,
                )

            def nlive(b, n):
                return sum(
                    1 for c in range(cb[b]) if info[(b, c, n)] is not None
                )

            def emit_chunk(b, c, u, u0, o_ps):
                st = [info[(b, c, 0)], info[(b, c, 1)]]
                kc = (offs[b] + c) * CHUNK
                qb = qbase[b]
                s_ps = ps_s.tile([CHUNK, QCOLS], F32, tag="s", name="s")
                for n in range(2):
                    if st[n] is None:
                        continue
                    a = n * NHALF + st[n]["qlo"]
                    z = (n + 1) * NHALF
                    mi = midx.get((b, c, n))
                    nc.tensor.matmul(
                        s_ps[:, a:z],
                        kt_t[:, kc:kc + CHUNK],
                        qt_t[:, qb + a:qb + z],
                        start=True, stop=mi is None,
                    )
                    if mi is not None:
                        blo, bhi, mo = mi
                        nc.tensor.matmul(
                            s_ps[:, n * NHALF + blo:n * NHALF + bhi],
                            identb_t[:],
                            masks_t[:, mo:mo + bhi - blo],
                            start=False, stop=True,
                        )
                alo = st[0]["qlo"] if st[0] is not None else NHALF + st[1]["qlo"]
                nc.scalar.activation(
                    u[:, alo:QCOLS], s_ps[:, alo:QCOLS], exp, scale=SCALE
                )
                if c == 1:
                    nc.vector.tensor_add(
                        lacc[b][:, alo:], u0[:, alo:], u[:, alo:]
                    )
                    # cols chunk 0 covers but chunk 1 does not (band edge)
                    for n in range(2):
                        if st[n] is not None and st[n]["qlo"] > 0:
                            gs, ge = n * NHALF, n * NHALF + st[n]["qlo"]
                            nc.scalar.copy(lacc[b][:, gs:ge], u0[:, gs:ge])
                elif c >= 2:
                    nc.vector.tensor_add(
                        lacc[b][:, alo:], lacc[b][:, alo:], u[:, alo:]
                    )
                if o_ps is not None:
                    for n in range(2):
                        if st[n] is None:
                            continue
                        a = n * NHALF + st[n]["qlo"]
                        z = (n + 1) * NHALF
                        nc.tensor.matmul(
                            o_ps[:, a:z],
                            v_t[:, kc:kc + CHUNK],
                            u[:, a:z],
                            start=c == 0, stop=c == last_n[b][n],
                        )

            def emit_lsum(b, u0):
                # broadcast column-sum: all-ones lhsT replicates the kv-sum
                # of l_acc into every PSUM partition
                lbc = ps_s.tile([CHUNK, QCOLS], F32, tag="s", name="lbc")
                for n in range(2):
                    hs = slice(n * NHALF, (n + 1) * NHALF)
                    src_t = lacc[b] if nlive(b, n) >= 2 else u0
                    nc.tensor.matmul(
                        lbc[:, hs], ones_t[:], src_t[:, hs],
                        start=True, stop=True,
                    )
                return lbc

            def emit_epilogue(eb, eo, u0):
                lbc = emit_lsum(eb, u0)
                rr = sbe.tile([CHUNK, QCOLS], F32, tag="rl", name="rl")
                nc.vector.reciprocal_approx_fast(rr[:], lbc[:])
                osb = sbe.tile([D, QCOLS], F32, tag="osb", name="osb")
                nc.vector.tensor_mul(osb[:], eo[:], rr[:])
                nc.sync.dma_start(out_d.ap()[eb][:, :], osb[:])

            # ---- phase A: shortest seq QK/exp only (PV deferred) ----
            u_f = []
            for c in range(cb[bf]):
                u = sbin.tile([CHUNK, QCOLS], F16, tag=f"uf{c}", name=f"uf{c}")
                emit_chunk(bf, c, u, u_f[0] if c else None, None)
                u_f.append(u)

            # ---- phase B: remaining seqs, PV interleaved per chunk ----
            rest = porder[1:]
            pend = None  # previous seq's deferred epilogue
            done_f = False
            for b in rest:
                o_ps = ps_o.tile([D, QCOLS], F32, tag="o", name="o")
                u0 = None
                for c in range(cb[b]):
                    if c == min(2, cb[b] - 1):
                        if not done_f:
                            lbc_f = emit_lsum(bf, u_f[0])
                            nc.vector.reciprocal_approx_fast(
                                rl_f[:], lbc_f[:]
                            )
                            done_f = True
                        if pend is not None:
                            emit_epilogue(*pend)
                            pend = None
                    uu = sbu.tile([CHUNK, QCOLS], F16, tag="u", name="u")
                    emit_chunk(b, c, uu, u0, o_ps)
                    if c == 0:
                        u0 = uu
                pend = (b, o_ps, u0)

            # ---- phase C: last epilogue + deferred PV of the shortest seq ----
            if pend is not None:
                emit_epilogue(*pend)
                pend = None
            o_ps = ps_o.tile([D, QCOLS], F32, tag="o", name="o")
            out_fsb = sbe.tile([D, QCOLS], F32, tag="osbf")
            for n in range(2):
                for c in range(cb[bf]):
                    st = info[(bf, c, n)]
                    if st is None:
                        continue
                    a = n * NHALF + st["qlo"]
                    z = (n + 1) * NHALF
                    nc.tensor.matmul(
                        o_ps[:, a:z],
                        v_t[:, (offs[bf] + c) * CHUNK:(offs[bf] + c + 1) * CHUNK],
                        u_f[c][:, a:z],
                        start=c == 0, stop=c == last_n[bf][n],
                    )
                hs = slice(n * NHALF, (n + 1) * NHALF)
                nc.vector.tensor_mul(
                    out_fsb[:, hs], o_ps[:, hs], rl_f[:, hs]
                )
                nc.sync.dma_start(out_d.ap()[bf][:, hs], out_fsb[:, hs])

    nc.compile()
    return nc, plan


def _pack_inputs(query, k_cache, v_cache, block_tables, plan):
    """Gather the paged cache and pack per-core fp16 shards."""
    L, cb, porder, offs, C = (
        plan["L"], plan["cb"], plan["porder"], plan["offs"], plan["C"]
    )
    k_lin = k_cache[block_tables].reshape(B, KV, KVH, D)
    v_lin = v_cache[block_tables].reshape(B, KV, KVH, D)
    kt_all = np.zeros((KVH, D, C * CHUNK), dtype=np.float16)
    v_all = np.zeros((KVH, CHUNK, C * CHUNK), dtype=np.float16)
    for b in range(B):
        Lb, w = int(L[b]), cb[b] * CHUNK
        o0 = offs[b] * CHUNK
        kk = np.zeros((w, KVH, D), dtype=np.float32)
        kk[:Lb] = k_lin[b, :Lb]
        kt_all[:, :, o0:o0 + w] = kk.transpose(1, 2, 0).astype(np.float16)
        vv = np.zeros((w, KVH, D), dtype=np.float32)
        vv[:Lb] = v_lin[b, :Lb]
        v_all[:, :, o0:o0 + w] = (
            vv.reshape(cb[b], CHUNK, KVH, D)
            .transpose(2, 1, 0, 3)
            .reshape(KVH, CHUNK, w)
            .astype(np.float16)
        )
    # query [B,Q,H,D] -> porder-major [KVH, D, B*QCOLS] (t-major, g inner)
    qp = query[np.array(porder)]
    qt_all = (
        qp.transpose(2, 3, 0, 1)
        .reshape(KVH, G, D, B, Q)
        .transpose(0, 2, 3, 4, 1)
        .reshape(KVH, D, B * QCOLS)
        .astype(np.float16)
    )
    return [
        {
            "kt": np.ascontiguousarray(kt_all[h]),
            "v": np.ascontiguousarray(v_all[h]),
            "qt": np.ascontiguousarray(qt_all[h]),
        }
        for h in range(KVH)
    ]


def _unpack_outputs(results):
    """[B,D,QCOLS] per core (O^T, q=(t,g) on cols) -> [B*Q, H*D]."""
    out = np.empty((B * Q, H * D), dtype=np.float32)
    for h, res in enumerate(results):
        o = res["out"].reshape(B, D, Q, G)  # [b, d, t, g]
        o = o.transpose(0, 2, 3, 1).reshape(B * Q, G * D)
        out[:, h * G * D:(h + 1) * G * D] = o
    return out


def kernel(query, k_cache, v_cache, block_tables, seq_lens):
    query = np.asarray(query, dtype=np.float32)
    k_cache = np.asarray(k_cache, dtype=np.float32)
    v_cache = np.asarray(v_cache, dtype=np.float32)
    block_tables = np.asarray(block_tables, dtype=np.int64)
    nc, plan = _build(np.asarray(seq_lens))
    in_maps = _pack_inputs(query, k_cache, v_cache, block_tables, plan)
    res = run_bass_kernel_spmd(nc, in_maps, core_ids=list(range(N_CORES)))
    return _unpack_outputs(res.results)


# revision 9
# speedup vs baseline: 1.3215x; 1.0629x over previous
"""Paged GQA chunked-prefill attention for 8 Trainium2 NeuronCores.

Problem (hardcoded): B=4 seqs x Q=256 new tokens, H=32 query heads, KVH=8 kv
heads (GQA group G=4), D=128 head dim, paged KV cache of 512 blocks x 16
tokens, per-seq lengths in seq_lens (clamped to >= Q), causal masking.

Sharding: tensor-parallel over heads. Core h gets kv head h and query heads
h*4..h*4+3; block_tables/seq_lens are resolved host-side while packing the
shards; the output is all-gathered host-side over the hidden dim.

v2 design (per core; q = (t, g) -> 1024 columns/seq; kv chunks of 128):
  S^T[kv,q] = K_c^T q          fp16 matmul into PSUM (full PE rate)
  u = exp(SCALE*S^T)           ScalarE, PSUM->SBUF, fp16 out
  mask                         multiplicative 0/1 fp16 band tiles on VectorE
  l_acc += u                   VectorE fp16 adds (4x perf mode) -- keeps the
                               denominator reduction OFF the PE
  O^T += V_c^T u               fp16 matmul, PSUM accumulation over chunks
Per-seq: partition_all_reduce(l_acc) on GpSimd -> l broadcast to all
partitions, reciprocal on VectorE, out = O^T * rl (GpSimd for interleaved
seqs, VectorE halves for the tail seq), DMA out.

Scheduling: the shortest seq runs QK/exp first (smallest DMA critical path)
but its PV runs LAST, so the tail after the final matmul is just one small
PV group + per-half multiply + DMA. Fully-masked query columns are skipped
(shrunk matmul/exp widths). 8 warm-up matmuls on a zeroed tile run during
the input DMA window so the PE HAM clock-gate is at 2.4 GHz when real work
arrives. DMA issues are spread over the SP/ACT/DVE/Pool sequencers.
"""
import math

import numpy as np

import concourse.mybir as mybir
import concourse.tile as tile
from concourse import bacc, bass_isa
from concourse.bass_utils import run_bass_kernel_spmd

B, Q, H, D = 4, 256, 32, 128
KVH = 8
G = H // KVH
BLOCK = 16
NB = 128
KV = NB * BLOCK
NUM_BLOCKS = B * NB
SCALE = 1.0 / math.sqrt(D)
N_CORES = 8
CHUNK = 128
QCOLS = G * Q  # 1024 q columns per sequence per core
NHALF = 512

F32 = mybir.dt.float32
F16 = mybir.dt.float16


def _plan(seq_lens):
    """Chunk counts, processing order, per-(seq,chunk,half) mask geometry."""
    L = np.maximum(np.asarray(seq_lens, dtype=np.int64), Q)
    cb = [int((int(x) + CHUNK - 1) // CHUNK) for x in L]
    first = min(range(B), key=lambda b: (cb[b], b))
    rest = sorted((b for b in range(B) if b != first), key=lambda b: (-cb[b], b))
    porder = [first] + rest
    offs = {}
    o = 0
    for b in porder:
        offs[b] = o
        o += cb[b]
    C = o
    # info[(b,c,n)]: None if the whole half is masked, else dict with
    # qlo (dead leading cols), blo/bhi (mask band col range within the half)
    info = {}
    for b in range(B):
        Lb = int(L[b])
        for c in range(cb[b]):
            for n in range(2):
                lo = Lb - Q + n * CHUNK  # qpos of this half's first column
                if c * CHUNK > lo + CHUNK - 1:
                    info[(b, c, n)] = None
                    continue
                tdead = min(max(c * CHUNK - lo, 0), CHUNK)
                thi = min(max(c * CHUNK + CHUNK - 1 - lo, 0), CHUNK)
                info[(b, c, n)] = dict(qlo=G * tdead, blo=G * tdead, bhi=G * thi)
    masks = []  # (b, c, n, tdead, thi, moff_t) in processing order
    moff = 0
    for b in porder:
        for c in range(cb[b]):
            for n in range(2):
                st = info[(b, c, n)]
                if st is None or st["bhi"] <= st["blo"]:
                    continue
                td, th = st["blo"] // G, st["bhi"] // G
                masks.append((b, c, n, td, th, moff))
                moff += th - td
    last_n = {
        b: [
            min(cb[b] - 1, (int(L[b]) - Q + n * CHUNK + CHUNK - 1) // CHUNK)
            for n in range(2)
        ]
        for b in range(B)
    }
    return dict(L=L, cb=cb, porder=porder, offs=offs, C=C, info=info,
                masks=masks, mtot=moff, last_n=last_n)


NEG = -20000.0  # exp(SCALE*(s+NEG)) underflows to exactly 0; fp16-exact


def _mask_np(plan):
    m = np.zeros((CHUNK, max(plan["mtot"], 1)), dtype=np.float16)
    p = np.arange(CHUNK)[:, None]
    for (b, c, n, td, th, moff) in plan["masks"]:
        lo = int(plan["L"][b]) - Q + n * CHUNK
        t = np.arange(td, th)[None, :]
        m[:, moff:moff + (th - td)] = np.where(
            c * CHUNK + p <= lo + t, 0.0, NEG
        ).astype(np.float16)
    return m


def _build(seq_lens):
    plan = _plan(seq_lens)
    L, cb, porder, offs = plan["L"], plan["cb"], plan["porder"], plan["offs"]
    C, info, mtot, last_n = plan["C"], plan["info"], plan["mtot"], plan["last_n"]
    midx = {(b, c, n): (td, th, mo)
            for (b, c, n, td, th, mo) in plan["masks"]}
    mask_np = _mask_np(plan)

    nc = bacc.Bacc(
        "TRN2", target_bir_lowering=False, debug=False, num_devices=N_CORES
    )
    kt_d = nc.dram_tensor("kt", [D, C * CHUNK], F16, kind="ExternalInput")
    v_d = nc.dram_tensor("v", [CHUNK, C * CHUNK], F16, kind="ExternalInput")
    qt_d = nc.dram_tensor("qt", [D, B * QCOLS], F16, kind="ExternalInput")
    out_d = nc.dram_tensor("out", [B, D, QCOLS], F32, kind="ExternalOutput")
    mask_d = nc.inline_tensor(mask_np, name="mask_const")
    identb_np = np.eye(CHUNK, dtype=np.float16)
    identb_d = nc.inline_tensor(identb_np, name="identb_const")

    exp = mybir.ActivationFunctionType.Exp
    bf = porder[0]
    qbase = {b: i * QCOLS for i, b in enumerate(porder)}

    def kvcols(b):
        return offs[b] * CHUNK, (offs[b] + cb[b]) * CHUNK

    with tile.TileContext(nc) as tc:
        with (
            tc.tile_pool(name="sbin", bufs=1) as sbin,
            tc.tile_pool(name="sbu", bufs=4) as sbu,
            tc.tile_pool(name="sbe", bufs=2) as sbe,
            tc.tile_pool(name="ps_s", bufs=2, space="PSUM") as ps_s,
            tc.tile_pool(name="ps_o", bufs=2, space="PSUM") as ps_o,
        ):
            kt_t = sbin.tile([D, C * CHUNK], F16, tag="kt")
            v_t = sbin.tile([CHUNK, C * CHUNK], F16, tag="v")
            qt_t = sbin.tile([D, B * QCOLS], F16, tag="qt")
            warm = sbin.tile([CHUNK, NHALF], F16, tag="warm")
            masks_t = sbin.tile([CHUNK, max(mtot, 1)], F16, tag="masks")
            lacc = {
                b: sbin.tile([CHUNK, QCOLS], F16, tag=f"lacc{b}", name=f"lacc{b}")
                for b in range(B)
            }
            rl_f = sbin.tile([CHUNK, QCOLS], F32, tag="rlf")
            identb_t = sbin.tile([CHUNK, CHUNK], F16, tag="identb")
            ones_t = sbin.tile([CHUNK, CHUNK], F16, tag="ones")

            nc.gpsimd.memset(warm[:], 0.0)
            nc.vector.memset(ones_t[:], 1.0)

            # ---- input DMAs, spread across sequencers ----
            # SP (fine-grained HWDGE): the PE-critical path, smallest first
            k0, k1 = kvcols(bf)
            qb0 = qbase[bf]
            nc.sync.dma_start(
                kt_t[:, k0:k0 + CHUNK], kt_d.ap()[:, k0:k0 + CHUNK]
            )
            nc.sync.dma_start(
                qt_t[:, qb0:qb0 + NHALF], qt_d.ap()[:, qb0:qb0 + NHALF]
            )
            if k1 > k0 + CHUNK:
                nc.sync.dma_start(
                    kt_t[:, k0 + CHUNK:k1], kt_d.ap()[:, k0 + CHUNK:k1]
                )
            nc.sync.dma_start(
                qt_t[:, qb0 + NHALF:qb0 + QCOLS],
                qt_d.ap()[:, qb0 + NHALF:qb0 + QCOLS],
            )
            s0 = k1
            sh = min(C * CHUNK, s0 + 2 * CHUNK)
            sb1 = min(C * CHUNK, s0 + 8 * CHUNK)
            nc.sync.dma_start(kt_t[:, s0:sh], kt_d.ap()[:, s0:sh])
            if sb1 > sh:
                nc.sync.dma_start(kt_t[:, sh:sb1], kt_d.ap()[:, sh:sb1])
            # ACT ring: masks/ident first (needed by the first chunks)
            nc.scalar.dma_start(identb_t[:], identb_d.ap())
            if mtot:
                nc.scalar.dma_start(masks_t[:], mask_d.ap())
            if B * QCOLS > QCOLS:
                nc.scalar.dma_start(qt_t[:, QCOLS:], qt_d.ap()[:, QCOLS:])
            nc.scalar.dma_start(v_t[:, s0:sb1], v_d.ap()[:, s0:sb1])
            # Pool ring (coarse SWDGE drain): the late-needed tails
            cut = sb1
            while cut < C * CHUNK:
                hi = min(cut + 8 * CHUNK, C * CHUNK)
                nc.gpsimd.dma_start(kt_t[:, cut:hi], kt_d.ap()[:, cut:hi])
                cut = hi
            cut = sb1
            while cut < C * CHUNK:
                hi = min(cut + 8 * CHUNK, C * CHUNK)
                nc.gpsimd.dma_start(v_t[:, cut:hi], v_d.ap()[:, cut:hi])
                cut = hi
            # v of the deferred-PV seq: only needed in phase C
            nc.sync.dma_start(v_t[:, k0:k1], v_d.ap()[:, k0:k1])

            # ---- PE warm-up (HAM un-throttle) on zeroed tile ----
            wps = ps_s.tile([CHUNK, QCOLS], F32, tag="s")
            for _ in range(8):
                nc.tensor.matmul(
                    wps[:, 0:NHALF], warm[:, 0:CHUNK], warm[:],
                    start=True, stop=True,
                )

            def nlive(b, n):
                return sum(
                    1 for c in range(cb[b]) if info[(b, c, n)] is not None
                )

            def emit_chunk(b, c, u, u0, o_ps):
                st = [info[(b, c, 0)], info[(b, c, 1)]]
                kc = (offs[b] + c) * CHUNK
                qb = qbase[b]
                s_ps = ps_s.tile([CHUNK, QCOLS], F32, tag="s", name="s")
                for n in range(2):
                    if st[n] is None:
                        continue
                    a = n * NHALF + st[n]["qlo"]
                    z = (n + 1) * NHALF
                    mi = midx.get((b, c, n))
                    nc.tensor.matmul(
                        s_ps[:, a:z],
                        kt_t[:, kc:kc + CHUNK],
                        qt_t[:, qb + a:qb + z],
                        start=True, stop=mi is None,
                    )
                    if mi is not None:
                        td, th, mo = mi
                        mb = (
                            masks_t[:, mo:mo + th - td]
                            .unsqueeze(2)
                            .broadcast_to([CHUNK, th - td, G])
                        )
                        nc.tensor.matmul(
                            s_ps[:, n * NHALF + G * td:n * NHALF + G * th],
                            identb_t[:], mb,
                            start=False, stop=True,
                        )
                alo = st[0]["qlo"] if st[0] is not None else NHALF + st[1]["qlo"]
                nc.scalar.activation(
                    u[:, alo:QCOLS], s_ps[:, alo:QCOLS], exp, scale=SCALE
                )
                if c == 1:
                    nc.vector.tensor_add(
                        lacc[b][:, alo:], u0[:, alo:], u[:, alo:]
                    )
                    # cols chunk 0 covers but chunk 1 does not (band edge)
                    for n in range(2):
                        if st[n] is not None and st[n]["qlo"] > 0:
                            gs, ge = n * NHALF, n * NHALF + st[n]["qlo"]
                            nc.scalar.copy(lacc[b][:, gs:ge], u0[:, gs:ge])
                elif c >= 2:
                    nc.vector.tensor_add(
                        lacc[b][:, alo:], lacc[b][:, alo:], u[:, alo:]
                    )
                if o_ps is not None:
                    for n in range(2):
                        if st[n] is None:
                            continue
                        a = n * NHALF + st[n]["qlo"]
                        z = (n + 1) * NHALF
                        nc.tensor.matmul(
                            o_ps[:, a:z],
                            v_t[:, kc:kc + CHUNK],
                            u[:, a:z],
                            start=c == 0, stop=c == last_n[b][n],
                        )

            def emit_lsum(b, u0):
                # broadcast column-sum: all-ones lhsT replicates the kv-sum
                # of l_acc into every PSUM partition
                lbc = ps_s.tile([CHUNK, QCOLS], F32, tag="s", name="lbc")
                for n in range(2):
                    hs = slice(n * NHALF, (n + 1) * NHALF)
                    src_t = lacc[b] if nlive(b, n) >= 2 else u0
                    nc.tensor.matmul(
                        lbc[:, hs], ones_t[:], src_t[:, hs],
                        start=True, stop=True,
                    )
                return lbc

            def emit_epilogue(eb, eo, u0):
                lbc = emit_lsum(eb, u0)
                rr = sbe.tile([CHUNK, QCOLS], F32, tag="rl", name="rl")
                nc.vector.reciprocal_approx_fast(rr[:], lbc[:])
                osb = sbe.tile([D, QCOLS], F32, tag="osb", name="osb")
                nc.vector.tensor_mul(osb[:], eo[:], rr[:])
                nc.sync.dma_start(out_d.ap()[eb][:, :], osb[:])

            # ---- phase A: shortest seq QK/exp only (PV deferred) ----
            u_f = []
            for c in range(cb[bf]):
                u = sbin.tile([CHUNK, QCOLS], F16, tag=f"uf{c}", name=f"uf{c}")
                emit_chunk(bf, c, u, u_f[0] if c else None, None)
                u_f.append(u)

            # ---- phase B: remaining seqs, PV interleaved per chunk ----
            rest = porder[1:]
            pend = None  # previous seq's deferred epilogue
            done_f = False
            for b in rest:
                o_ps = ps_o.tile([D, QCOLS], F32, tag="o", name="o")
                u0 = None
                for c in range(cb[b]):
                    if c == min(2, cb[b] - 1):
                        if not done_f:
                            lbc_f = emit_lsum(bf, u_f[0])
                            nc.vector.reciprocal_approx_fast(
                                rl_f[:], lbc_f[:]
                            )
                            done_f = True
                        if pend is not None:
                            emit_epilogue(*pend)
                            pend = None
                    uu = sbu.tile([CHUNK, QCOLS], F16, tag="u", name="u")
                    emit_chunk(b, c, uu, u0, o_ps)
                    if c == 0:
                        u0 = uu
                pend = (b, o_ps, u0)

            # ---- phase C: last epilogue + deferred PV of the shortest seq ----
            if pend is not None:
                emit_epilogue(*pend)
                pend = None
            o_ps = ps_o.tile([D, QCOLS], F32, tag="o", name="o")
            out_fsb = sbe.tile([D, QCOLS], F32, tag="osbf")
            for n in range(2):
                for c in range(cb[bf]):
                    st = info[(bf, c, n)]
                    if st is None:
                        continue
                    a = n * NHALF + st["qlo"]
                    z = (n + 1) * NHALF
                    nc.tensor.matmul(
                        o_ps[:, a:z],
                        v_t[:, (offs[bf] + c) * CHUNK:(offs[bf] + c + 1) * CHUNK],
                        u_f[c][:, a:z],
                        start=c == 0, stop=c == last_n[bf][n],
                    )
                hs = slice(n * NHALF, (n + 1) * NHALF)
                nc.vector.tensor_mul(
                    out_fsb[:, hs], o_ps[:, hs], rl_f[:, hs]
                )
                nc.sync.dma_start(out_d.ap()[bf][:, hs], out_fsb[:, hs])

    nc.compile()
    return nc, plan


def _pack_inputs(query, k_cache, v_cache, block_tables, plan):
    """Gather the paged cache and pack per-core fp16 shards."""
    L, cb, porder, offs, C = (
        plan["L"], plan["cb"], plan["porder"], plan["offs"], plan["C"]
    )
    k_lin = k_cache[block_tables].reshape(B, KV, KVH, D)
    v_lin = v_cache[block_tables].reshape(B, KV, KVH, D)
    kt_all = np.zeros((KVH, D, C * CHUNK), dtype=np.float16)
    v_all = np.zeros((KVH, CHUNK, C * CHUNK), dtype=np.float16)
    for b in range(B):
        Lb, w = int(L[b]), cb[b] * CHUNK
        o0 = offs[b] * CHUNK
        kk = np.zeros((w, KVH, D), dtype=np.float32)
        kk[:Lb] = k_lin[b, :Lb]
        kt_all[:, :, o0:o0 + w] = kk.transpose(1, 2, 0).astype(np.float16)
        vv = np.zeros((w, KVH, D), dtype=np.float32)
        vv[:Lb] = v_lin[b, :Lb]
        v_all[:, :, o0:o0 + w] = (
            vv.reshape(cb[b], CHUNK, KVH, D)
            .transpose(2, 1, 0, 3)
            .reshape(KVH, CHUNK, w)
            .astype(np.float16)
        )
    # query [B,Q,H,D] -> porder-major [KVH, D, B*QCOLS] (t-major, g inner)
    qp = query[np.array(porder)]
    qt_all = (
        qp.transpose(2, 3, 0, 1)
        .reshape(KVH, G, D, B, Q)
        .transpose(0, 2, 3, 4, 1)
        .reshape(KVH, D, B * QCOLS)
        .astype(np.float16)
    )
    return [
        {
            "kt": np.ascontiguousarray(kt_all[h]),
            "v": np.ascontiguousarray(v_all[h]),
            "qt": np.ascontiguousarray(qt_all[h]),
        }
        for h in range(KVH)
    ]


def _unpack_outputs(results):
    """[B,D,QCOLS] per core (O^T, q=(t,g) on cols) -> [B*Q, H*D]."""
    out = np.empty((B * Q, H * D), dtype=np.float32)
    for h, res in enumerate(results):
        o = res["out"].reshape(B, D, Q, G)  # [b, d, t, g]
        o = o.transpose(0, 2, 3, 1).reshape(B * Q, G * D)
        out[:, h * G * D:(h + 1) * G * D] = o
    return out


def kernel(query, k_cache, v_cache, block_tables, seq_lens):
    query = np.asarray(query, dtype=np.float32)
    k_cache = np.asarray(k_cache, dtype=np.float32)
    v_cache = np.asarray(v_cache, dtype=np.float32)
    block_tables = np.asarray(block_tables, dtype=np.int64)
    nc, plan = _build(np.asarray(seq_lens))
    in_maps = _pack_inputs(query, k_cache, v_cache, block_tables, plan)
    res = run_bass_kernel_spmd(nc, in_maps, core_ids=list(range(N_CORES)))
    return _unpack_outputs(res.results)


# revision 10
# speedup vs baseline: 1.3609x; 1.0298x over previous
"""Paged GQA chunked-prefill attention for 8 Trainium2 NeuronCores.

Problem (hardcoded): B=4 seqs x Q=256 new tokens, H=32 query heads, KVH=8 kv
heads (GQA group G=4), D=128 head dim, paged KV cache of 512 blocks x 16
tokens, per-seq lengths in seq_lens (clamped to >= Q), causal masking.

Sharding: tensor-parallel over heads. Core h gets kv head h and query heads
h*4..h*4+3; block_tables/seq_lens are resolved host-side while packing the
shards; the output is all-gathered host-side over the hidden dim.

v2 design (per core; q = (t, g) -> 1024 columns/seq; kv chunks of 128):
  S^T[kv,q] = K_c^T q          fp16 matmul into PSUM (full PE rate)
  u = exp(SCALE*S^T)           ScalarE, PSUM->SBUF, fp16 out
  mask                         multiplicative 0/1 fp16 band tiles on VectorE
  l_acc += u                   VectorE fp16 adds (4x perf mode) -- keeps the
                               denominator reduction OFF the PE
  O^T += V_c^T u               fp16 matmul, PSUM accumulation over chunks
Per-seq: partition_all_reduce(l_acc) on GpSimd -> l broadcast to all
partitions, reciprocal on VectorE, out = O^T * rl (GpSimd for interleaved
seqs, VectorE halves for the tail seq), DMA out.

Scheduling: the shortest seq runs QK/exp first (smallest DMA critical path)
but its PV runs LAST, so the tail after the final matmul is just one small
PV group + per-half multiply + DMA. Fully-masked query columns are skipped
(shrunk matmul/exp widths). 8 warm-up matmuls on a zeroed tile run during
the input DMA window so the PE HAM clock-gate is at 2.4 GHz when real work
arrives. DMA issues are spread over the SP/ACT/DVE/Pool sequencers.
"""
import math

import numpy as np

import concourse.mybir as mybir
import concourse.tile as tile
from concourse import bacc, bass_isa
from concourse.bass_utils import run_bass_kernel_spmd

B, Q, H, D = 4, 256, 32, 128
KVH = 8
G = H // KVH
BLOCK = 16
NB = 128
KV = NB * BLOCK
NUM_BLOCKS = B * NB
SCALE = 1.0 / math.sqrt(D)
N_CORES = 8
CHUNK = 128
QCOLS = G * Q  # 1024 q columns per sequence per core
NHALF = 512

F32 = mybir.dt.float32
F16 = mybir.dt.float16


def _plan(seq_lens):
    """Chunk counts, processing order, per-(seq,chunk,half) mask geometry."""
    L = np.maximum(np.asarray(seq_lens, dtype=np.int64), Q)
    cb = [int((int(x) + CHUNK - 1) // CHUNK) for x in L]
    first = min(range(B), key=lambda b: (cb[b], b))
    rest = sorted((b for b in range(B) if b != first), key=lambda b: (-cb[b], b))
    porder = [first] + rest
    offs = {}
    o = 0
    for b in porder:
        offs[b] = o
        o += cb[b]
    C = o
    # info[(b,c,n)]: None if the whole half is masked, else dict with
    # qlo (dead leading cols), blo/bhi (mask band col range within the half)
    info = {}
    for b in range(B):
        Lb = int(L[b])
        for c in range(cb[b]):
            for n in range(2):
                lo = Lb - Q + n * CHUNK  # qpos of this half's first column
                if c * CHUNK > lo + CHUNK - 1:
                    info[(b, c, n)] = None
                    continue
                tdead = min(max(c * CHUNK - lo, 0), CHUNK)
                thi = min(max(c * CHUNK + CHUNK - 1 - lo, 0), CHUNK)
                info[(b, c, n)] = dict(qlo=G * tdead, blo=G * tdead, bhi=G * thi)
    masks = []  # (b, c, n, tdead, thi, moff_t) in processing order
    moff = 0
    for b in porder:
        for c in range(cb[b]):
            for n in range(2):
                st = info[(b, c, n)]
                if st is None or st["bhi"] <= st["blo"]:
                    continue
                td, th = st["blo"] // G, st["bhi"] // G
                masks.append((b, c, n, td, th, moff))
                moff += th - td
    last_n = {
        b: [
            min(cb[b] - 1, (int(L[b]) - Q + n * CHUNK + CHUNK - 1) // CHUNK)
            for n in range(2)
        ]
        for b in range(B)
    }
    return dict(L=L, cb=cb, porder=porder, offs=offs, C=C, info=info,
                masks=masks, mtot=moff, last_n=last_n)


NEG = -20000.0  # exp(SCALE*(s+NEG)) underflows to exactly 0; fp16-exact


def _mask_np(plan):
    m = np.zeros((CHUNK, max(plan["mtot"], 1)), dtype=np.float16)
    p = np.arange(CHUNK)[:, None]
    for (b, c, n, td, th, moff) in plan["masks"]:
        lo = int(plan["L"][b]) - Q + n * CHUNK
        t = np.arange(td, th)[None, :]
        m[:, moff:moff + (th - td)] = np.where(
            c * CHUNK + p <= lo + t, 0.0, NEG
        ).astype(np.float16)
    return m


def _build(seq_lens):
    plan = _plan(seq_lens)
    L, cb, porder, offs = plan["L"], plan["cb"], plan["porder"], plan["offs"]
    C, info, mtot, last_n = plan["C"], plan["info"], plan["mtot"], plan["last_n"]
    midx = {(b, c, n): (td, th, mo)
            for (b, c, n, td, th, mo) in plan["masks"]}
    mask_np = _mask_np(plan)

    nc = bacc.Bacc(
        "TRN2", target_bir_lowering=False, debug=False, num_devices=N_CORES
    )
    kt_d = nc.dram_tensor("kt", [D, C * CHUNK], F16, kind="ExternalInput")
    v_d = nc.dram_tensor("v", [CHUNK, C * CHUNK], F16, kind="ExternalInput")
    qt_d = nc.dram_tensor("qt", [D, B * QCOLS], F16, kind="ExternalInput")
    out_d = nc.dram_tensor("out", [B, D, QCOLS], F16, kind="ExternalOutput")
    mask_d = nc.inline_tensor(mask_np, name="mask_const")
    identb_np = np.eye(CHUNK, dtype=np.float16)
    identb_d = nc.inline_tensor(identb_np, name="identb_const")

    exp = mybir.ActivationFunctionType.Exp
    bf = porder[0]
    qbase = {b: i * QCOLS for i, b in enumerate(porder)}

    def kvcols(b):
        return offs[b] * CHUNK, (offs[b] + cb[b]) * CHUNK

    with tile.TileContext(nc) as tc:
        with (
            tc.tile_pool(name="sbin", bufs=1) as sbin,
            tc.tile_pool(name="sbu", bufs=4) as sbu,
            tc.tile_pool(name="sbe", bufs=2) as sbe,
            tc.tile_pool(name="ps_s", bufs=2, space="PSUM") as ps_s,
            tc.tile_pool(name="ps_o", bufs=2, space="PSUM") as ps_o,
        ):
            kt_t = sbin.tile([D, C * CHUNK], F16, tag="kt")
            v_t = sbin.tile([CHUNK, C * CHUNK], F16, tag="v")
            qt_t = sbin.tile([D, B * QCOLS], F16, tag="qt")
            warm = sbin.tile([CHUNK, NHALF], F16, tag="warm")
            masks_t = sbin.tile([CHUNK, max(mtot, 1)], F16, tag="masks")
            lacc = {
                b: sbin.tile([CHUNK, QCOLS], F16, tag=f"lacc{b}", name=f"lacc{b}")
                for b in range(B)
            }
            rl_f = sbin.tile([CHUNK, QCOLS], F32, tag="rlf")
            identb_t = sbin.tile([CHUNK, CHUNK], F16, tag="identb")
            ones_t = sbin.tile([CHUNK, CHUNK], F16, tag="ones")

            nc.gpsimd.memset(warm[:], 0.0)
            nc.vector.memset(ones_t[:], 1.0)

            # ---- input DMAs, spread across sequencers ----
            # SP (fine-grained HWDGE): the PE-critical path, smallest first
            k0, k1 = kvcols(bf)
            qb0 = qbase[bf]
            nc.sync.dma_start(
                kt_t[:, k0:k0 + CHUNK], kt_d.ap()[:, k0:k0 + CHUNK]
            )
            nc.sync.dma_start(
                qt_t[:, qb0:qb0 + NHALF], qt_d.ap()[:, qb0:qb0 + NHALF]
            )
            if k1 > k0 + CHUNK:
                nc.sync.dma_start(
                    kt_t[:, k0 + CHUNK:k1], kt_d.ap()[:, k0 + CHUNK:k1]
                )
            nc.sync.dma_start(
                qt_t[:, qb0 + NHALF:qb0 + QCOLS],
                qt_d.ap()[:, qb0 + NHALF:qb0 + QCOLS],
            )
            s0 = k1
            sh = min(C * CHUNK, s0 + 2 * CHUNK)
            sb1 = min(C * CHUNK, s0 + 8 * CHUNK)
            nc.sync.dma_start(kt_t[:, s0:sh], kt_d.ap()[:, s0:sh])
            if sb1 > sh:
                nc.sync.dma_start(kt_t[:, sh:sb1], kt_d.ap()[:, sh:sb1])
            # ACT ring: masks/ident first (needed by the first chunks)
            nc.scalar.dma_start(identb_t[:], identb_d.ap())
            if mtot:
                nc.scalar.dma_start(masks_t[:], mask_d.ap())
            if B * QCOLS > QCOLS:
                nc.scalar.dma_start(qt_t[:, QCOLS:], qt_d.ap()[:, QCOLS:])
            nc.scalar.dma_start(v_t[:, s0:sb1], v_d.ap()[:, s0:sb1])
            # Pool ring (coarse SWDGE drain): the late-needed tails
            cut = sb1
            while cut < C * CHUNK:
                hi = min(cut + 8 * CHUNK, C * CHUNK)
                nc.gpsimd.dma_start(kt_t[:, cut:hi], kt_d.ap()[:, cut:hi])
                cut = hi
            cut = sb1
            while cut < C * CHUNK:
                hi = min(cut + 8 * CHUNK, C * CHUNK)
                nc.gpsimd.dma_start(v_t[:, cut:hi], v_d.ap()[:, cut:hi])
                cut = hi
            # v of the deferred-PV seq: only needed in phase C
            nc.sync.dma_start(v_t[:, k0:k1], v_d.ap()[:, k0:k1])

            # ---- PE warm-up (HAM un-throttle) on zeroed tile ----
            wps = ps_s.tile([CHUNK, QCOLS], F32, tag="s")
            for _ in range(12):
                nc.tensor.matmul(
                    wps[:, 0:NHALF], warm[:, 0:CHUNK], warm[:],
                    start=True, stop=True,
                )

            def nlive(b, n):
                return sum(
                    1 for c in range(cb[b]) if info[(b, c, n)] is not None
                )

            def emit_chunk(b, c, u, u0):
                st = [info[(b, c, 0)], info[(b, c, 1)]]
                kc = (offs[b] + c) * CHUNK
                qb = qbase[b]
                s_ps = ps_s.tile([CHUNK, QCOLS], F32, tag="s", name="s")
                for n in range(2):
                    if st[n] is None:
                        continue
                    a = n * NHALF + st[n]["qlo"]
                    z = (n + 1) * NHALF
                    mi = midx.get((b, c, n))
                    nc.tensor.matmul(
                        s_ps[:, a:z],
                        kt_t[:, kc:kc + CHUNK],
                        qt_t[:, qb + a:qb + z],
                        start=True, stop=mi is None,
                    )
                    if mi is not None:
                        td, th, mo = mi
                        mb = (
                            masks_t[:, mo:mo + th - td]
                            .unsqueeze(2)
                            .broadcast_to([CHUNK, th - td, G])
                        )
                        nc.tensor.matmul(
                            s_ps[:, n * NHALF + G * td:n * NHALF + G * th],
                            identb_t[:], mb,
                            start=False, stop=True,
                        )
                alo = st[0]["qlo"] if st[0] is not None else NHALF + st[1]["qlo"]
                nc.scalar.activation(
                    u[:, alo:QCOLS], s_ps[:, alo:QCOLS], exp, scale=SCALE
                )
                if c == 1:
                    nc.vector.tensor_add(
                        lacc[b][:, alo:], u0[:, alo:], u[:, alo:]
                    )
                    # cols chunk 0 covers but chunk 1 does not (band edge)
                    for n in range(2):
                        if st[n] is not None and st[n]["qlo"] > 0:
                            gs, ge = n * NHALF, n * NHALF + st[n]["qlo"]
                            nc.scalar.copy(lacc[b][:, gs:ge], u0[:, gs:ge])
                elif c >= 2:
                    nc.vector.tensor_add(
                        lacc[b][:, alo:], lacc[b][:, alo:], u[:, alo:]
                    )

            def emit_pv(b, c, u, o_ps):
                st = [info[(b, c, 0)], info[(b, c, 1)]]
                kc = (offs[b] + c) * CHUNK
                for n in range(2):
                    if st[n] is None:
                        continue
                    a = n * NHALF + st[n]["qlo"]
                    z = (n + 1) * NHALF
                    nc.tensor.matmul(
                        o_ps[:, a:z],
                        v_t[:, kc:kc + CHUNK],
                        u[:, a:z],
                        start=c == 0, stop=c == last_n[b][n],
                    )

            def emit_lsum(b, u0):
                # broadcast column-sum: all-ones lhsT replicates the kv-sum
                # of l_acc into every PSUM partition
                lbc = ps_s.tile([CHUNK, QCOLS], F32, tag="s", name="lbc")
                for n in range(2):
                    hs = slice(n * NHALF, (n + 1) * NHALF)
                    src_t = lacc[b] if nlive(b, n) >= 2 else u0
                    nc.tensor.matmul(
                        lbc[:, hs], ones_t[:], src_t[:, hs],
                        start=True, stop=True,
                    )
                return lbc

            def emit_epilogue(eb, eo, u0):
                lbc = emit_lsum(eb, u0)
                rr = sbe.tile([CHUNK, QCOLS], F32, tag="rl", name="rl")
                nc.vector.reciprocal_approx_fast(rr[:], lbc[:])
                osb = sbe.tile([D, QCOLS], F16, tag="osb", name="osb")
                nc.vector.tensor_mul(osb[:], eo[:], rr[:])
                nc.sync.dma_start(out_d.ap()[eb][:, :], osb[:])

            # ---- phase A: shortest seq QK/exp only (PV deferred) ----
            u_f = []
            for c in range(cb[bf]):
                u = sbin.tile([CHUNK, QCOLS], F16, tag=f"uf{c}", name=f"uf{c}")
                emit_chunk(bf, c, u, u_f[0] if c else None)
                u_f.append(u)

            # ---- phase B: remaining seqs, PV interleaved per chunk ----
            rest = porder[1:]
            pend = None  # previous seq's deferred epilogue
            done_f = False
            for b in rest:
                o_ps = ps_o.tile([D, QCOLS], F32, tag="o", name="o")
                u0 = None
                u_prev = None
                for c in range(cb[b]):
                    if c == min(2, cb[b] - 1):
                        if not done_f:
                            lbc_f = emit_lsum(bf, u_f[0])
                            nc.vector.reciprocal_approx_fast(
                                rl_f[:], lbc_f[:]
                            )
                            done_f = True
                        if pend is not None:
                            emit_epilogue(*pend)
                            pend = None
                    uu = sbu.tile([CHUNK, QCOLS], F16, tag="u", name="u")
                    emit_chunk(b, c, uu, u0)
                    if c > 0:
                        emit_pv(b, c - 1, u_prev, o_ps)
                    u_prev = uu
                    if c == 0:
                        u0 = uu
                emit_pv(b, cb[b] - 1, u_prev, o_ps)
                pend = (b, o_ps, u0)

            # ---- phase C: last epilogue + deferred PV of the shortest seq ----
            if pend is not None:
                emit_epilogue(*pend)
                pend = None
            o_ps = ps_o.tile([D, QCOLS], F32, tag="o", name="o")
            out_fsb = sbe.tile([D, QCOLS], F16, tag="osbf")
            for n in range(2):
                for c in range(cb[bf]):
                    st = info[(bf, c, n)]
                    if st is None:
                        continue
                    a = n * NHALF + st["qlo"]
                    z = (n + 1) * NHALF
                    nc.tensor.matmul(
                        o_ps[:, a:z],
                        v_t[:, (offs[bf] + c) * CHUNK:(offs[bf] + c + 1) * CHUNK],
                        u_f[c][:, a:z],
                        start=c == 0, stop=c == last_n[bf][n],
                    )
                hs = slice(n * NHALF, (n + 1) * NHALF)
                nc.vector.tensor_mul(
                    out_fsb[:, hs], o_ps[:, hs], rl_f[:, hs]
                )
                nc.sync.dma_start(out_d.ap()[bf][:, hs], out_fsb[:, hs])

    nc.compile()
    return nc, plan


def _pack_inputs(query, k_cache, v_cache, block_tables, plan):
    """Gather the paged cache and pack per-core fp16 shards."""
    L, cb, porder, offs, C = (
        plan["L"], plan["cb"], plan["porder"], plan["offs"], plan["C"]
    )
    k_lin = k_cache[block_tables].reshape(B, KV, KVH, D)
    v_lin = v_cache[block_tables].reshape(B, KV, KVH, D)
    kt_all = np.zeros((KVH, D, C * CHUNK), dtype=np.float16)
    v_all = np.zeros((KVH, CHUNK, C * CHUNK), dtype=np.float16)
    for b in range(B):
        Lb, w = int(L[b]), cb[b] * CHUNK
        o0 = offs[b] * CHUNK
        kk = np.zeros((w, KVH, D), dtype=np.float32)
        kk[:Lb] = k_lin[b, :Lb]
        kt_all[:, :, o0:o0 + w] = kk.transpose(1, 2, 0).astype(np.float16)
        vv = np.zeros((w, KVH, D), dtype=np.float32)
        vv[:Lb] = v_lin[b, :Lb]
        v_all[:, :, o0:o0 + w] = (
            vv.reshape(cb[b], CHUNK, KVH, D)
            .transpose(2, 1, 0, 3)
            .reshape(KVH, CHUNK, w)
            .astype(np.float16)
        )
    # query [B,Q,H,D] -> porder-major [KVH, D, B*QCOLS] (t-major, g inner)
    qp = query[np.array(porder)]
    qt_all = (
        qp.transpose(2, 3, 0, 1)
        .reshape(KVH, G, D, B, Q)
        .transpose(0, 2, 3, 4, 1)
        .reshape(KVH, D, B * QCOLS)
        .astype(np.float16)
    )
    return [
        {
            "kt": np.ascontiguousarray(kt_all[h]),
            "v": np.ascontiguousarray(v_all[h]),
            "qt": np.ascontiguousarray(qt_all[h]),
        }
        for h in range(KVH)
    ]


def _unpack_outputs(results):
    """[B,D,QCOLS] per core (O^T, q=(t,g) on cols) -> [B*Q, H*D]."""
    out = np.empty((B * Q, H * D), dtype=np.float32)
    for h, res in enumerate(results):
        o = res["out"].reshape(B, D, Q, G)  # [b, d, t, g]
        o = o.transpose(0, 2, 3, 1).reshape(B * Q, G * D)
        out[:, h * G * D:(h + 1) * G * D] = o
    return out


def kernel(query, k_cache, v_cache, block_tables, seq_lens):
    query = np.asarray(query, dtype=np.float32)
    k_cache = np.asarray(k_cache, dtype=np.float32)
    v_cache = np.asarray(v_cache, dtype=np.float32)
    block_tables = np.asarray(block_tables, dtype=np.int64)
    nc, plan = _build(np.asarray(seq_lens))
    in_maps = _pack_inputs(query, k_cache, v_cache, block_tables, plan)
    res = run_bass_kernel_spmd(nc, in_maps, core_ids=list(range(N_CORES)))
    return _unpack_outputs(res.results)
